# revision 7
# baseline (speedup 1.0000x reference)
"""Trainium2 Bass kernel for nn_AutoregressiveLAMDecoder (B=16384, D=1024, H=8, NT=4, NC=16).

Data-parallel over 8 cores (R=2048 rows/core). Exact algebraic restructure:
  - cross-attention collapses (softmax over one key): ca = mem @ (Wo_ca@Wv_ca).T + b
  - the whole self-attention block output + token/pos embedding depends only on
    the discrete token prefix (<= 4096 combos per position): precomputed on host
    into per-position BASE tables, gathered per row on device (transposed layout)
  - ff_w2/out_w fold into a (2048, 16) matrix; ln1/ln3 affines fold into weights
  - layernorms computed in the transposed (feature-partition) layout via
    ones-vector matmul reductions; no 128x128 transposes anywhere
All matmuls bf16 with fp32 PSUM accumulation; stats and softmax f32 on host.
"""
import sys
for _p in ('/opt/trn_rl_repo', '/root/.axon_site/_ro/trn_rl_repo'):
    if _p not in sys.path:
        sys.path.insert(0, _p)

import math
import numpy as np
import ml_dtypes

B, D, H = 16384, 1024, 8
NT, NC = 4, 16
DFF = 2048
DH = D // H
N_CORES = 8
R = B // N_CORES          # rows per core
BF16 = ml_dtypes.bfloat16

_CACHE = {}


# ---------------------------------------------------------------- host math
def _ln_rows(x, g, b, eps=1e-5):
    m = x.mean(-1, keepdims=True)
    v = ((x - m) ** 2).mean(-1, keepdims=True)
    return (x - m) / np.sqrt(v + eps) * g + b


def _host_precompute(i):
    """Weight-only folds and per-combo BASE tables (float64/float32)."""
    f = {k: np.asarray(v, np.float64) for k, v in i.items()
         if np.asarray(v).dtype not in (np.int64, np.int32)}
    P = {}
    P['WcpT'] = (f['cp_w'] * f['cp_ln_g'][None, :]).T            # (din, dout)
    P['b_cp'] = f['cp_b'] + f['cp_w'] @ f['cp_ln_b']
    W_ca = f['ca_wo'] @ f['ca_wv']
    P['WcaT'] = W_ca.T
    P['b_ca'] = f['ca_wo'] @ f['ca_bv'] + f['ca_bo']
    P['W2pT'] = (f['out_w'] @ f['ff_w2']).T                       # (2048, 16)
    P['b_out2'] = f['out_b'] + f['out_w'] @ f['ff_b2']
    P['W1T'] = (f['ff_w1'] * f['ln3_g'][None, :]).T               # (1024, 2048)
    P['b1'] = f['ff_b1'] + f['ff_w1'] @ f['ln3_b']
    P['OwT'] = f['out_w'].T                                       # (1024, 16)

    E = np.stack([f['tok_emb'] + f['pos_emb'][p][None, :] for p in range(NT)])
    L = np.stack([_ln_rows(E[p], f['ln1_g'], f['ln1_b']) for p in range(NT)])
    Q = (L @ f['sa_wq'].T + f['sa_bq']).reshape(NT, NC + 1, H, DH)
    K = (L @ f['sa_wk'].T + f['sa_bk']).reshape(NT, NC + 1, H, DH)
    V = (L @ f['sa_wv'].T + f['sa_bv']).reshape(NT, NC + 1, H, DH)

    # BASE_p[combo] = E[p][qtok] + SA_out(combo) @ wo.T + bo, where combo
    # encodes targets t_0..t_{p-1} (position p attends to shifted[0..p]).
    bases = []
    for p in range(NT):
        S = NC ** p
        digits = np.arange(S)
        ctoks = np.empty((S, p + 1), np.int64)
        ctoks[:, 0] = NC                                      # start token
        for j in range(1, p + 1):
            ctoks[:, j] = (digits // (NC ** (p - j))) % NC    # t_{j-1}
        qtok = ctoks[:, p]
        # scores s[n, h, j] = Q[p][qtok]·K[j][ctoks_j] / sqrt(dh)
        s = np.empty((S, H, p + 1))
        for j in range(p + 1):
            s[:, :, j] = np.einsum('nhd,nhd->nh', Q[p][qtok],
                                   K[j][ctoks[:, j]]) / math.sqrt(DH)
        s -= s.max(-1, keepdims=True)
        a = np.exp(s)
        a /= a.sum(-1, keepdims=True)
        o = np.zeros((S, H, DH))
        for j in range(p + 1):
            o += a[:, :, j:j+1] * V[j][ctoks[:, j]]
        sa = o.reshape(S, D) @ f['sa_wo'].T + f['sa_bo']
        bases.append((E[p][qtok] + sa).astype(np.float32))
    P['base0'] = bases[0][0]                                  # (1024,)
    P['base1'] = bases[1]                                     # (16, 1024)
    P['base2'] = bases[2]                                     # (256, 1024)
    P['base3'] = bases[3]                                     # (4096, 1024)
    return P


def _shared_inputs(P):
    bf = lambda a: np.ascontiguousarray(a, BF16)
    f32 = lambda a: np.ascontiguousarray(a, np.float32)
    col = lambda b, n: f32(np.asarray(b).reshape(n, 128).T)   # [128, n]
    return {
        'wcp': bf(P['WcpT']), 'wca': bf(P['WcaT']),
        'w1': bf(P['W1T']), 'w2p': bf(P['W2pT']), 'oww': bf(P['OwT']),
        'base1': bf(P['base1']), 'base2': bf(P['base2']),
        'base3': bf(P['base3']),
        'bcp': col(P['b_cp'], 8),
        'bca': col(P['b_ca'], 8),
        'b0c': col(P['base0'], 8),
        'bb1': col(P['b1'], 16),
        'bout': f32(np.asarray(P['b_out2']).reshape(16, 1)),
    }


def _per_core_inputs(P, ctx_shard, tg_shard):
    """Batch-dependent marshalling for one core."""
    r = ctx_shard.shape[0]
    t0 = tg_shard[:, 0].astype(np.int64)
    t1 = tg_shard[:, 1].astype(np.int64)
    t2 = tg_shard[:, 2].astype(np.int64)
    idxs = {'gi1': t0, 'gi2': t0 * 16 + t1, 'gi3': t0 * 256 + t1 * 16 + t2}
    out = {'ctxT': np.ascontiguousarray(ctx_shard.T, dtype=BF16)}
    s16 = np.arange(32) * 16
    for k, idx in idxs.items():
        w = np.zeros((128, r // 16), np.int16)
        for c in range(r // 512):
            blk = idx[c*512:(c+1)*512]
            for q in range(128):
                w[q, c*32:(c+1)*32] = blk[s16 + q % 16]
        out[k] = w
    return out


def make_in_maps(inputs):
    ctx_full = np.asarray(inputs['context'], np.float32)
    tg_full = np.asarray(inputs['targets']).astype(np.int64)
    P = _host_precompute(inputs)
    shared = _shared_inputs(P)
    in_maps = []
    for c in range(N_CORES):
        m = dict(shared)
        m.update(_per_core_inputs(P, ctx_full[c*R:(c+1)*R],
                                  tg_full[c*R:(c+1)*R]))
        in_maps.append(m)
    return in_maps


# ---------------------------------------------------------------- device build
def build_nc(rows=R, rep=1):
    import concourse.bass as bass
    import concourse.mybir as mybir
    from concourse import bacc
    from concourse.tile import TileContext

    dt = mybir.dt
    AF = mybir.ActivationFunctionType
    OP = mybir.AluOpType

    NCH = rows // 512

    nc = bacc.Bacc("TRN2", target_bir_lowering=False, debug=False,
                   num_devices=N_CORES)
    din = lambda n, s, d: nc.dram_tensor(n, s, d, kind="ExternalInput").ap()
    ctxT_d = din("ctxT", [D, rows], dt.bfloat16)
    gi1_d = din("gi1", [128, rows // 16], dt.int16)
    gi2_d = din("gi2", [128, rows // 16], dt.int16)
    gi3_d = din("gi3", [128, rows // 16], dt.int16)
    wcp_d = din("wcp", [D, D], dt.bfloat16)
    wca_d = din("wca", [D, D], dt.bfloat16)
    w1_d = din("w1", [D, DFF], dt.bfloat16)
    w2p_d = din("w2p", [DFF, 16], dt.bfloat16)
    ow_d = din("oww", [D, 16], dt.bfloat16)
    base1_d = din("base1", [16, D], dt.bfloat16)
    base2_d = din("base2", [256, D], dt.bfloat16)
    base3_d = din("base3", [4096, D], dt.bfloat16)
    bcp_d = din("bcp", [128, 8], dt.float32)
    bca_d = din("bca", [128, 8], dt.float32)
    b0c_d = din("b0c", [128, 8], dt.float32)
    bb1_d = din("bb1", [128, 16], dt.float32)
    bout_d = din("bout", [16, 1], dt.float32)
    out_d = nc.dram_tensor("out", [NT, 16, rows], dt.float32,
                           kind="ExternalOutput").ap()
    base_d = [None, base1_d, base2_d, base3_d]
    gi_d = [None, gi1_d, gi2_d, gi3_d]

    with TileContext(nc) as tc:
        with (
            tc.tile_pool(name="wp", bufs=1) as wp,
            tc.tile_pool(name="bt", bufs=3) as btp,
            tc.tile_pool(name="fm", bufs=2) as fm,
            tc.tile_pool(name="rl", bufs=2) as rl,
            tc.tile_pool(name="st", bufs=2) as st,
            tc.tile_pool(name="pmm", bufs=5, space="PSUM") as pmm,
            tc.tile_pool(name="pst", bufs=2, space="PSUM") as pst,
            tc.tile_pool(name="pO", bufs=1, space="PSUM") as pO,
        ):
            # ---- constants / weights
            ones_k = wp.tile([128, 1], dt.bfloat16, tag="onesk")
            nc.vector.memset(ones_k, 1.0)
            ones_m = wp.tile([1, 128], dt.bfloat16, tag="onesm")
            nc.vector.memset(ones_m, 1.0)
            eps1 = wp.tile([1, 1], dt.float32, tag="eps1")
            nc.vector.memset(eps1, 1e-5)

            wcp = wp.tile([128, 8, D], dt.bfloat16, tag="wcp")
            nc.sync.dma_start(wcp[:], wcp_d.rearrange("(k p) n -> p k n", p=128))
            wca = wp.tile([128, 8, D], dt.bfloat16, tag="wca")
            nc.sync.dma_start(wca[:], wca_d.rearrange("(k p) n -> p k n", p=128))
            w1 = wp.tile([128, 8, DFF], dt.bfloat16, tag="w1")
            nc.sync.dma_start(w1[:], w1_d.rearrange("(k p) n -> p k n", p=128))
            w2p = wp.tile([128, 16, 16], dt.bfloat16, tag="w2p")
            nc.sync.dma_start(w2p[:], w2p_d.rearrange("(k p) n -> p k n", p=128))
            oww = wp.tile([128, 8, 16], dt.bfloat16, tag="oww")
            nc.sync.dma_start(oww[:], ow_d.rearrange("(k p) n -> p k n", p=128))
            bcp = wp.tile([128, 8], dt.float32, tag="bcp")
            nc.sync.dma_start(bcp[:], bcp_d[:])
            bca = wp.tile([128, 8], dt.float32, tag="bca")
            nc.sync.dma_start(bca[:], bca_d[:])
            b0c = wp.tile([128, 8], dt.float32, tag="b0c")
            nc.sync.dma_start(b0c[:], b0c_d[:])
            bb1 = wp.tile([128, 16], dt.float32, tag="bb1")
            nc.sync.dma_start(bb1[:], bb1_d[:])
            bout = wp.tile([16, 1], dt.float32, tag="bout")
            nc.sync.dma_start(bout[:], bout_d[:])
            gi = [None] * 4
            for p in (1, 2, 3):
                gtile = wp.tile([128, rows // 16], dt.int16, tag=f"gi{p}")
                nc.sync.dma_start(gtile[:], gi_d[p][:])
                gi[p] = gtile

            def row_stats(src_tiles, sq_tag):
                """src: list of 8 [128,512] bf16 tiles (or [128,8,512] views).
                Returns (mu_b, nmu_b, rs_b) bf16 [128,512] broadcast tiles."""
                sps = pst.tile([1, 512], dt.float32, tag="stat")
                qps = pst.tile([1, 512], dt.float32, tag="stat")
                for kb in range(8):
                    xt = src_tiles(kb)
                    nc.tensor.matmul(sps[:], ones_k[:], xt,
                                     start=(kb == 0), stop=(kb == 7))
                    sq = rl.tile([128, 512], dt.bfloat16, tag=sq_tag, bufs=2)
                    nc.vector.tensor_tensor(sq[:], xt, xt, OP.mult)
                    nc.tensor.matmul(qps[:], ones_k[:], sq[:],
                                     start=(kb == 0), stop=(kb == 7))
                mean = st.tile([1, 512], dt.float32, tag="statf", bufs=3)
                nc.scalar.activation(mean[:], sps[:], AF.Copy, bias=0.0,
                                     scale=1.0 / D)
                meanb = st.tile([1, 512], dt.bfloat16, tag="statb", bufs=2)
                nc.vector.tensor_copy(meanb[:], mean[:])
                m2 = st.tile([1, 512], dt.float32, tag="statf", bufs=3)
                nc.vector.tensor_tensor(m2[:], mean[:], mean[:], OP.mult)
                var = st.tile([1, 512], dt.float32, tag="statf", bufs=3)
                nc.vector.scalar_tensor_tensor(
                    out=var[:], in0=qps[:], scalar=1.0 / D,
                    in1=m2[:], op0=OP.mult, op1=OP.subtract)
                lv = st.tile([1, 512], dt.float32, tag="statf", bufs=3)
                nc.scalar.activation(lv[:], var[:], AF.Ln, bias=eps1[:])
                rs = st.tile([1, 512], dt.bfloat16, tag="statb", bufs=2)
                nc.scalar.activation(rs[:], lv[:], AF.Exp, bias=0.0,
                                     scale=-0.5)
                mbc = pmm.tile([128, 512], dt.float32, tag="mm")
                nc.tensor.matmul(mbc[:], ones_m[:], meanb[:], start=True,
                                 stop=True)
                rbc = pmm.tile([128, 512], dt.float32, tag="mm")
                nc.tensor.matmul(rbc[:], ones_m[:], rs[:], start=True,
                                 stop=True)
                mu_b = st.tile([128, 512], dt.bfloat16, tag="mub", bufs=2)
                nc.scalar.copy(mu_b[:], mbc[:])
                rs_b = st.tile([128, 512], dt.bfloat16, tag="rsb", bufs=2)
                nc.scalar.copy(rs_b[:], rbc[:])
                return mu_b, rs_b

            for chn in range(NCH):
                c0 = chn * 512
                # ---- load ctxT slice, LN via matmul stats
                xt = fm.tile([128, 8, 512], dt.bfloat16, tag="xt")
                for kb in range(8):
                    nc.sync.dma_start(
                        xt[:, kb, :], ctxT_d[kb*128:(kb+1)*128, c0:c0+512])
                mu_b, rs_b = row_stats(lambda kb: xt[:, kb, :], "sqc")
                lnx = fm.tile([128, 8, 512], dt.bfloat16, tag="lnx")
                for kb in range(8):
                    t = rl.tile([128, 512], dt.bfloat16, tag="t", bufs=2)
                    nc.vector.tensor_tensor(t[:], xt[:, kb, :], mu_b[:],
                                            OP.subtract)
                    nc.vector.tensor_tensor(lnx[:, kb, :], t[:], rs_b[:],
                                            OP.mult)
                # ---- mem = gelu(cp(lnx))
                mem = fm.tile([128, 8, 512], dt.bfloat16, tag="mem", bufs=1)
                for mb in range(8):
                    z = pmm.tile([128, 512], dt.float32, tag="mm")
                    for kb in range(8):
                        nc.tensor.matmul(z[:], wcp[:, kb, mb*128:(mb+1)*128],
                                         lnx[:, kb, :],
                                         start=(kb == 0), stop=(kb == 7))
                    nc.scalar.activation(mem[:, mb, :], z[:], AF.Gelu,
                                         bias=bcp[:, mb:mb+1])
                # ---- ca = Wca @ mem + bca
                casb = fm.tile([128, 8, 512], dt.bfloat16, tag="ca")
                for mb in range(8):
                    z = pmm.tile([128, 512], dt.float32, tag="mm")
                    for kb in range(8):
                        nc.tensor.matmul(z[:], wca[:, kb, mb*128:(mb+1)*128],
                                         mem[:, kb, :],
                                         start=(kb == 0), stop=(kb == 7))
                    nc.scalar.activation(casb[:, mb, :], z[:], AF.Identity,
                                         bias=bca[:, mb:mb+1])

                for p in range(NT):
                    # ---- x2 = base_p(combo) + ca
                    x2 = fm.tile([128, 8, 512], dt.bfloat16, tag="x2", bufs=1)
                    if p == 0:
                        for kb in range(8):
                            nc.vector.tensor_scalar(
                                x2[:, kb, :], casb[:, kb, :],
                                b0c[:, kb:kb+1], None, OP.add)
                    else:
                        bt = btp.tile([128, 8, 512], dt.bfloat16, tag="bt")
                        nc.gpsimd.dma_gather(
                            out_ap=bt[:],
                            in_ap=base_d[p],
                            idxs_ap=gi[p][:, chn*32:(chn+1)*32],
                            num_idxs=512,
                            num_idxs_reg=512,
                            elem_size=D,
                            transpose=True,
                        )
                        for kb in range(8):
                            nc.vector.tensor_tensor(
                                x2[:, kb, :], bt[:, kb, :], casb[:, kb, :],
                                OP.add)
                    # ---- ln3 stats + normalize
                    mu3, rs3 = row_stats(lambda kb: x2[:, kb, :], "sq2")
                    x2n = fm.tile([128, 8, 512], dt.bfloat16, tag="x2n", bufs=1)
                    for kb in range(8):
                        t = rl.tile([128, 512], dt.bfloat16, tag="t", bufs=2)
                        nc.vector.tensor_tensor(t[:], x2[:, kb, :], mu3[:],
                                                OP.subtract)
                        nc.vector.tensor_tensor(x2n[:, kb, :], t[:], rs3[:],
                                                OP.mult)
                    # ---- out = x2 @ Ow + relu(x2n @ W1 + b1) @ W2p + bout
                    Ops = pO.tile([16, 512], dt.float32, tag="O")
                    for kb in range(8):
                        nc.tensor.matmul(Ops[:], oww[:, kb, :], x2[:, kb, :],
                                         start=(kb == 0), stop=False)
                    for fb in range(16):
                        hps = pmm.tile([128, 512], dt.float32, tag="mm")
                        for kb in range(8):
                            nc.tensor.matmul(
                                hps[:], w1[:, kb, fb*128:(fb+1)*128],
                                x2n[:, kb, :],
                                start=(kb == 0), stop=(kb == 7))
                        hsb = rl.tile([128, 512], dt.bfloat16, tag="hsb",
                                      bufs=2)
                        nc.scalar.activation(hsb[:], hps[:], AF.Relu,
                                             bias=bb1[:, fb:fb+1])
                        nc.tensor.matmul(Ops[:], w2p[:, fb, :], hsb[:],
                                         start=False, stop=(fb == 15))
                    Osb = rl.tile([16, 512], dt.float32, tag="Osb", bufs=2)
                    nc.scalar.activation(Osb[:], Ops[:], AF.Identity,
                                         bias=bout[:, 0:1])
                    nc.sync.dma_start(out_d[p, :, c0:c0+512], Osb[:])

    nc.compile()
    return nc


# ---------------------------------------------------------------- PJRT runner
class _SpmdRunner:
    def __init__(self, nc, n_cores):
        import jax
        import numpy as _np
        from jax.sharding import Mesh, PartitionSpec
        from jax.experimental.shard_map import shard_map
        import concourse.mybir as mybir
        from concourse import bass2jax
        bass2jax.install_neuronx_cc_hook()
        self.jax = jax
        self.n_cores = n_cores
        partition_name = (nc.partition_id_tensor.name
                          if nc.partition_id_tensor else None)
        in_names, out_names, out_avals, zero_outs = [], [], [], []
        for alloc in nc.m.functions[0].allocations:
            if not isinstance(alloc, mybir.MemoryLocationSet):
                continue
            name = alloc.memorylocations[0].name
            if alloc.kind == "ExternalInput":
                if name != partition_name:
                    in_names.append(name)
            elif alloc.kind == "ExternalOutput":
                shape = tuple(alloc.tensor_shape)
                dtype = mybir.dt.np(alloc.dtype)
                out_names.append(name)
                out_avals.append(jax.core.ShapedArray(shape, dtype))
                zero_outs.append(_np.zeros(shape, dtype))
        self.in_names, self.out_names = in_names, out_names
        self.out_avals, self.zero_outs = out_avals, zero_outs
        n_params, n_outs = len(in_names), len(out_avals)
        all_in = in_names + out_names
        if partition_name is not None:
            all_in.append(partition_name)

        def _body(*args):
            operands = list(args)
            if partition_name is not None:
                operands.append(bass2jax.partition_id_tensor())
            return tuple(bass2jax._bass_exec_p.bind(
                *operands, out_avals=tuple(out_avals),
                in_names=tuple(all_in), out_names=tuple(out_names),
                lowering_input_output_aliases=(),
                sim_require_finite=True, sim_require_nnan=True, nc=nc))

        devices = jax.devices()[:n_cores]
        mesh = Mesh(_np.asarray(devices), ("core",))
        self.sharded = jax.jit(
            shard_map(_body, mesh=mesh,
                      in_specs=(PartitionSpec("core"),) * (n_params + n_outs),
                      out_specs=(PartitionSpec("core"),) * n_outs,
                      check_rep=False),
            donate_argnums=tuple(range(n_params, n_params + n_outs)),
            keep_unused=True)

    def concat_inputs(self, in_maps):
        import numpy as _np
        per_core = [[_np.asarray(m[n]) for n in self.in_names] for m in in_maps]
        return [_np.concatenate([per_core[c][i] for c in range(self.n_cores)], 0)
                for i in range(len(self.in_names))]

    def zeros(self):
        import numpy as _np
        return [_np.zeros((self.n_cores * z.shape[0], *z.shape[1:]), z.dtype)
                for z in self.zero_outs]

    def run_concat(self, concat_in):
        out_arrs = self.sharded(*concat_in, *self.zeros())
        import numpy as _np
        return [_np.asarray(a) for a in out_arrs]


def _get_runner(rows=R, rep=1):
    key = (rows, rep)
    if key not in _CACHE:
        nc = build_nc(rows, rep)
        _CACHE[key] = _SpmdRunner(nc, N_CORES)
    return _CACHE[key]


# ---------------------------------------------------------------- public entry
def kernel(**inputs):
    ctx_full = np.asarray(inputs['context'], np.float32)
    assert ctx_full.shape == (B, D)
    runner = _get_runner(R, 1)
    in_maps = make_in_maps(inputs)
    concat_in = runner.concat_inputs(in_maps)
    outs = runner.run_concat(concat_in)
    o = outs[0].reshape(N_CORES, NT, 16, R)
    logits = np.ascontiguousarray(o.transpose(0, 3, 1, 2)).reshape(B, NT, 16)
    return logits.astype(np.float32)


# revision 9
# speedup vs baseline: 1.1770x; 1.1770x over previous
"""Trainium2 Bass kernel for nn_AutoregressiveLAMDecoder (B=16384, D=1024, H=8, NT=4, NC=16).

Data-parallel over 8 cores (R=2048 rows/core). Exact algebraic restructure:
  - cross-attention collapses (softmax over one key): ca = mem @ (Wo_ca@Wv_ca).T + b
  - the whole self-attention block output + token/pos embedding depends only on
    the discrete token prefix (<= 4096 combos per position): precomputed on host
    into per-position BASE tables, gathered per row on device (transposed layout)
  - ff_w2/out_w fold into a (2048, 16) matrix; ln1/ln3 affines fold into weights
  - layernorms computed in the transposed (feature-partition) layout via
    ones-vector matmul reductions; no 128x128 transposes anywhere
All matmuls bf16 with fp32 PSUM accumulation; stats and softmax f32 on host.
"""
import sys
for _p in ('/opt/trn_rl_repo', '/root/.axon_site/_ro/trn_rl_repo'):
    if _p not in sys.path:
        sys.path.insert(0, _p)

import math
import numpy as np
import ml_dtypes

B, D, H = 16384, 1024, 8
NT, NC = 4, 16
DFF = 2048
DH = D // H
N_CORES = 8
R = B // N_CORES          # rows per core
BF16 = ml_dtypes.bfloat16

_CACHE = {}


# ---------------------------------------------------------------- host math
def _ln_rows(x, g, b, eps=1e-5):
    m = x.mean(-1, keepdims=True)
    v = ((x - m) ** 2).mean(-1, keepdims=True)
    return (x - m) / np.sqrt(v + eps) * g + b


def _host_precompute(i):
    """Weight-only folds and per-combo BASE tables (float64/float32)."""
    f = {k: np.asarray(v, np.float64) for k, v in i.items()
         if np.asarray(v).dtype not in (np.int64, np.int32)}
    P = {}
    P['WcpT'] = (f['cp_w'] * f['cp_ln_g'][None, :]).T            # (din, dout)
    P['b_cp'] = f['cp_b'] + f['cp_w'] @ f['cp_ln_b']
    W_ca = f['ca_wo'] @ f['ca_wv']
    P['WcaT'] = W_ca.T
    P['b_ca'] = f['ca_wo'] @ f['ca_bv'] + f['ca_bo']
    P['W2pT'] = (f['out_w'] @ f['ff_w2']).T                       # (2048, 16)
    P['b_out2'] = f['out_b'] + f['out_w'] @ f['ff_b2']
    P['W1T'] = (f['ff_w1'] * f['ln3_g'][None, :]).T               # (1024, 2048)
    P['b1'] = f['ff_b1'] + f['ff_w1'] @ f['ln3_b']
    P['OwT'] = f['out_w'].T                                       # (1024, 16)

    E = np.stack([f['tok_emb'] + f['pos_emb'][p][None, :] for p in range(NT)])
    L = np.stack([_ln_rows(E[p], f['ln1_g'], f['ln1_b']) for p in range(NT)])
    Q = (L @ f['sa_wq'].T + f['sa_bq']).reshape(NT, NC + 1, H, DH)
    K = (L @ f['sa_wk'].T + f['sa_bk']).reshape(NT, NC + 1, H, DH)
    V = (L @ f['sa_wv'].T + f['sa_bv']).reshape(NT, NC + 1, H, DH)

    # BASE_p[combo] = E[p][qtok] + SA_out(combo) @ wo.T + bo, where combo
    # encodes targets t_0..t_{p-1} (position p attends to shifted[0..p]).
    bases = []
    for p in range(NT):
        S = NC ** p
        digits = np.arange(S)
        ctoks = np.empty((S, p + 1), np.int64)
        ctoks[:, 0] = NC                                      # start token
        for j in range(1, p + 1):
            ctoks[:, j] = (digits // (NC ** (p - j))) % NC    # t_{j-1}
        qtok = ctoks[:, p]
        # scores s[n, h, j] = Q[p][qtok]·K[j][ctoks_j] / sqrt(dh)
        s = np.empty((S, H, p + 1))
        for j in range(p + 1):
            s[:, :, j] = np.einsum('nhd,nhd->nh', Q[p][qtok],
                                   K[j][ctoks[:, j]]) / math.sqrt(DH)
        s -= s.max(-1, keepdims=True)
        a = np.exp(s)
        a /= a.sum(-1, keepdims=True)
        o = np.zeros((S, H, DH))
        for j in range(p + 1):
            o += a[:, :, j:j+1] * V[j][ctoks[:, j]]
        sa = o.reshape(S, D) @ f['sa_wo'].T + f['sa_bo']
        bases.append((E[p][qtok] + sa).astype(np.float32))
    P['base0'] = bases[0][0]                                  # (1024,)
    P['base1'] = bases[1]                                     # (16, 1024)
    P['base2'] = bases[2]                                     # (256, 1024)
    P['base3'] = bases[3]                                     # (4096, 1024)
    return P


def _shared_inputs(P):
    bf = lambda a: np.ascontiguousarray(a, BF16)
    f32 = lambda a: np.ascontiguousarray(a, np.float32)
    col = lambda b, n: f32(np.asarray(b).reshape(n, 128).T)   # [128, n]
    return {
        'wcp': bf(P['WcpT']), 'wca': bf(P['WcaT']),
        'w1': bf(P['W1T']), 'w2p': bf(P['W2pT']), 'oww': bf(P['OwT']),
        'base1': bf(P['base1']), 'base2': bf(P['base2']),
        'base3': bf(P['base3']),
        'bcp': col(P['b_cp'], 8),
        'bca': col(P['b_ca'], 8),
        'b0c': col(P['base0'], 8),
        'bb1': col(P['b1'], 16),
        'bout': f32(np.asarray(P['b_out2']).reshape(16, 1)),
    }


def _per_core_inputs(P, ctx_shard, tg_shard):
    """Batch-dependent marshalling for one core."""
    r = ctx_shard.shape[0]
    t0 = tg_shard[:, 0].astype(np.int64)
    t1 = tg_shard[:, 1].astype(np.int64)
    t2 = tg_shard[:, 2].astype(np.int64)
    idxs = {'gi1': t0, 'gi2': t0 * 16 + t1, 'gi3': t0 * 256 + t1 * 16 + t2}
    out = {'ctxT': np.ascontiguousarray(ctx_shard.T, dtype=BF16)}
    s16 = np.arange(32) * 16
    for k, idx in idxs.items():
        w = np.zeros((128, r // 16), np.int16)
        for c in range(r // 512):
            blk = idx[c*512:(c+1)*512]
            for q in range(128):
                w[q, c*32:(c+1)*32] = blk[s16 + q % 16]
        out[k] = w
    return out


def make_in_maps(inputs):
    ctx_full = np.asarray(inputs['context'], np.float32)
    tg_full = np.asarray(inputs['targets']).astype(np.int64)
    P = _host_precompute(inputs)
    shared = _shared_inputs(P)
    in_maps = []
    for c in range(N_CORES):
        m = dict(shared)
        m.update(_per_core_inputs(P, ctx_full[c*R:(c+1)*R],
                                  tg_full[c*R:(c+1)*R]))
        in_maps.append(m)
    return in_maps


# ---------------------------------------------------------------- device build
def build_nc(rows=R, rep=1):
    import concourse.bass as bass
    import concourse.mybir as mybir
    from concourse import bacc
    from concourse.tile import TileContext

    dt = mybir.dt
    AF = mybir.ActivationFunctionType
    OP = mybir.AluOpType

    NCH = rows // 512

    nc = bacc.Bacc("TRN2", target_bir_lowering=False, debug=False,
                   num_devices=N_CORES)
    din = lambda n, s, d: nc.dram_tensor(n, s, d, kind="ExternalInput").ap()
    ctxT_d = din("ctxT", [D, rows], dt.bfloat16)
    gi1_d = din("gi1", [128, rows // 16], dt.int16)
    gi2_d = din("gi2", [128, rows // 16], dt.int16)
    gi3_d = din("gi3", [128, rows // 16], dt.int16)
    wcp_d = din("wcp", [D, D], dt.bfloat16)
    wca_d = din("wca", [D, D], dt.bfloat16)
    w1_d = din("w1", [D, DFF], dt.bfloat16)
    w2p_d = din("w2p", [DFF, 16], dt.bfloat16)
    ow_d = din("oww", [D, 16], dt.bfloat16)
    base1_d = din("base1", [16, D], dt.bfloat16)
    base2_d = din("base2", [256, D], dt.bfloat16)
    base3_d = din("base3", [4096, D], dt.bfloat16)
    bcp_d = din("bcp", [128, 8], dt.float32)
    bca_d = din("bca", [128, 8], dt.float32)
    b0c_d = din("b0c", [128, 8], dt.float32)
    bb1_d = din("bb1", [128, 16], dt.float32)
    bout_d = din("bout", [16, 1], dt.float32)
    out_d = nc.dram_tensor("out", [NT, 16, rows], dt.float32,
                           kind="ExternalOutput").ap()
    base_d = [None, base1_d, base2_d, base3_d]
    gi_d = [None, gi1_d, gi2_d, gi3_d]

    with TileContext(nc) as tc:
        with (
            tc.tile_pool(name="wp", bufs=1) as wp,
            tc.tile_pool(name="bt", bufs=3) as btp,
            tc.tile_pool(name="fm", bufs=2) as fm,
            tc.tile_pool(name="rl", bufs=2) as rl,
            tc.tile_pool(name="st", bufs=2) as st,
            tc.tile_pool(name="pmm", bufs=5, space="PSUM") as pmm,
            tc.tile_pool(name="pst", bufs=2, space="PSUM") as pst,
            tc.tile_pool(name="pO", bufs=1, space="PSUM") as pO,
        ):
            # ---- constants / weights
            ones_k = wp.tile([128, 1], dt.bfloat16, tag="onesk")
            nc.vector.memset(ones_k, 1.0)
            ones_m = wp.tile([1, 128], dt.bfloat16, tag="onesm")
            nc.vector.memset(ones_m, 1.0)
            eps1 = wp.tile([1, 1], dt.float32, tag="eps1")
            nc.vector.memset(eps1, 1e-5)

            wcp = wp.tile([128, 8, D], dt.bfloat16, tag="wcp")
            nc.sync.dma_start(wcp[:], wcp_d.rearrange("(k p) n -> p k n", p=128))
            wca = wp.tile([128, 8, D], dt.bfloat16, tag="wca")
            nc.sync.dma_start(wca[:], wca_d.rearrange("(k p) n -> p k n", p=128))
            w1 = wp.tile([128, 8, DFF], dt.bfloat16, tag="w1")
            nc.sync.dma_start(w1[:], w1_d.rearrange("(k p) n -> p k n", p=128))
            w2p = wp.tile([128, 16, 16], dt.bfloat16, tag="w2p")
            nc.sync.dma_start(w2p[:], w2p_d.rearrange("(k p) n -> p k n", p=128))
            oww = wp.tile([128, 8, 16], dt.bfloat16, tag="oww")
            nc.sync.dma_start(oww[:], ow_d.rearrange("(k p) n -> p k n", p=128))
            bcp = wp.tile([128, 8], dt.float32, tag="bcp")
            nc.sync.dma_start(bcp[:], bcp_d[:])
            bca = wp.tile([128, 8], dt.float32, tag="bca")
            nc.sync.dma_start(bca[:], bca_d[:])
            b0c = wp.tile([128, 8], dt.float32, tag="b0c")
            nc.sync.dma_start(b0c[:], b0c_d[:])
            bb1 = wp.tile([128, 16], dt.float32, tag="bb1")
            nc.sync.dma_start(bb1[:], bb1_d[:])
            bout = wp.tile([16, 1], dt.float32, tag="bout")
            nc.sync.dma_start(bout[:], bout_d[:])
            gi = [None] * 4
            for p in (1, 2, 3):
                gtile = wp.tile([128, rows // 16], dt.int16, tag=f"gi{p}")
                nc.sync.dma_start(gtile[:], gi_d[p][:])
                gi[p] = gtile

            def row_stats(src_tiles, sq_tag):
                """src: list of 8 [128,512] bf16 tiles (or [128,8,512] views).
                Returns (mu_b, nmu_b, rs_b) bf16 [128,512] broadcast tiles."""
                sps = pst.tile([1, 512], dt.float32, tag="stat")
                qps = pst.tile([1, 512], dt.float32, tag="stat")
                for kb in range(8):
                    xt = src_tiles(kb)
                    nc.tensor.matmul(sps[:], ones_k[:], xt,
                                     start=(kb == 0), stop=(kb == 7))
                    sq = rl.tile([128, 512], dt.bfloat16, tag=sq_tag, bufs=2)
                    nc.vector.tensor_tensor(sq[:], xt, xt, OP.mult)
                    nc.tensor.matmul(qps[:], ones_k[:], sq[:],
                                     start=(kb == 0), stop=(kb == 7))
                mean = st.tile([1, 512], dt.float32, tag="statf", bufs=3)
                nc.scalar.activation(mean[:], sps[:], AF.Copy, bias=0.0,
                                     scale=1.0 / D)
                meanb = st.tile([1, 512], dt.bfloat16, tag="statb", bufs=2)
                nc.vector.tensor_copy(meanb[:], mean[:])
                m2 = st.tile([1, 512], dt.float32, tag="statf", bufs=3)
                nc.vector.tensor_tensor(m2[:], mean[:], mean[:], OP.mult)
                var = st.tile([1, 512], dt.float32, tag="statf", bufs=3)
                nc.vector.scalar_tensor_tensor(
                    out=var[:], in0=qps[:], scalar=1.0 / D,
                    in1=m2[:], op0=OP.mult, op1=OP.subtract)
                sd = st.tile([1, 512], dt.float32, tag="statf", bufs=3)
                nc.scalar.activation(sd[:], var[:], AF.Sqrt, bias=eps1[:])
                rs = st.tile([1, 512], dt.bfloat16, tag="statb", bufs=2)
                with nc.allow_low_precision(reason="rstd broadcast is bf16 anyway"):
                    nc.vector.reciprocal(rs[:], sd[:])
                mbc = pmm.tile([128, 512], dt.float32, tag="mm")
                nc.tensor.matmul(mbc[:], ones_m[:], meanb[:], start=True,
                                 stop=True)
                rbc = pmm.tile([128, 512], dt.float32, tag="mm")
                nc.tensor.matmul(rbc[:], ones_m[:], rs[:], start=True,
                                 stop=True)
                mu_b = st.tile([128, 512], dt.bfloat16, tag="mub", bufs=2)
                nc.scalar.copy(mu_b[:], mbc[:])
                rs_b = st.tile([128, 512], dt.bfloat16, tag="rsb", bufs=2)
                nc.scalar.copy(rs_b[:], rbc[:])
                return mu_b, rs_b

            for chn in range(NCH):
                c0 = chn * 512
                # ---- load ctxT slice, LN via matmul stats
                xt = fm.tile([128, 8, 512], dt.bfloat16, tag="xt")
                for kb in range(8):
                    nc.sync.dma_start(
                        xt[:, kb, :], ctxT_d[kb*128:(kb+1)*128, c0:c0+512])
                mu_b, rs_b = row_stats(lambda kb: xt[:, kb, :], "sqc")
                lnx = fm.tile([128, 8, 512], dt.bfloat16, tag="lnx")
                for kb in range(8):
                    t = rl.tile([128, 512], dt.bfloat16, tag="t", bufs=2)
                    nc.vector.tensor_tensor(t[:], xt[:, kb, :], mu_b[:],
                                            OP.subtract)
                    nc.vector.tensor_tensor(lnx[:, kb, :], t[:], rs_b[:],
                                            OP.mult)
                # ---- mem = gelu(cp(lnx))
                mem = fm.tile([128, 8, 512], dt.bfloat16, tag="mem", bufs=1)
                for mb in range(8):
                    z = pmm.tile([128, 512], dt.float32, tag="mm")
                    for kb in range(8):
                        nc.tensor.matmul(z[:], wcp[:, kb, mb*128:(mb+1)*128],
                                         lnx[:, kb, :],
                                         start=(kb == 0), stop=(kb == 7))
                    nc.scalar.activation(mem[:, mb, :], z[:], AF.Gelu,
                                         bias=bcp[:, mb:mb+1])
                # ---- ca = Wca @ mem + bca
                casb = fm.tile([128, 8, 512], dt.bfloat16, tag="ca")
                for mb in range(8):
                    z = pmm.tile([128, 512], dt.float32, tag="mm")
                    for kb in range(8):
                        nc.tensor.matmul(z[:], wca[:, kb, mb*128:(mb+1)*128],
                                         mem[:, kb, :],
                                         start=(kb == 0), stop=(kb == 7))
                    nc.scalar.activation(casb[:, mb, :], z[:], AF.Identity,
                                         bias=bca[:, mb:mb+1])

                for p in range(NT):
                    # ---- x2 = base_p(combo) + ca
                    x2 = fm.tile([128, 8, 512], dt.bfloat16, tag="x2", bufs=1)
                    if p == 0:
                        for kb in range(8):
                            nc.vector.tensor_scalar(
                                x2[:, kb, :], casb[:, kb, :],
                                b0c[:, kb:kb+1], None, OP.add)
                    else:
                        bt = btp.tile([128, 8, 512], dt.bfloat16, tag="bt")
                        nc.gpsimd.dma_gather(
                            out_ap=bt[:],
                            in_ap=base_d[p],
                            idxs_ap=gi[p][:, chn*32:(chn+1)*32],
                            num_idxs=512,
                            num_idxs_reg=512,
                            elem_size=D,
                            transpose=True,
                        )
                        for kb in range(8):
                            nc.vector.tensor_tensor(
                                x2[:, kb, :], bt[:, kb, :], casb[:, kb, :],
                                OP.add)
                    # ---- ln3 stats + normalize
                    mu3, rs3 = row_stats(lambda kb: x2[:, kb, :], "sq2")
                    x2n = fm.tile([128, 8, 512], dt.bfloat16, tag="x2n", bufs=1)
                    for kb in range(8):
                        t = rl.tile([128, 512], dt.bfloat16, tag="t", bufs=2)
                        nc.vector.tensor_tensor(t[:], x2[:, kb, :], mu3[:],
                                                OP.subtract)
                        nc.vector.tensor_tensor(x2n[:, kb, :], t[:], rs3[:],
                                                OP.mult)
                    # ---- out = x2 @ Ow + relu(x2n @ W1 + b1) @ W2p + bout
                    Ops = pO.tile([16, 512], dt.float32, tag="O")
                    for kb in range(8):
                        nc.tensor.matmul(Ops[:], oww[:, kb, :], x2[:, kb, :],
                                         start=(kb == 0), stop=False)
                    for fb in range(16):
                        hps = pmm.tile([128, 512], dt.float32, tag="mm")
                        for kb in range(8):
                            nc.tensor.matmul(
                                hps[:], w1[:, kb, fb*128:(fb+1)*128],
                                x2n[:, kb, :],
                                start=(kb == 0), stop=(kb == 7))
                        hsb = rl.tile([128, 512], dt.bfloat16, tag="hsb",
                                      bufs=2)
                        nc.scalar.activation(hsb[:], hps[:], AF.Relu,
                                             bias=bb1[:, fb:fb+1])
                        nc.tensor.matmul(Ops[:], w2p[:, fb, :], hsb[:],
                                         start=False, stop=(fb == 15))
                    Osb = rl.tile([16, 512], dt.float32, tag="Osb", bufs=2)
                    nc.scalar.activation(Osb[:], Ops[:], AF.Identity,
                                         bias=bout[:, 0:1])
                    nc.sync.dma_start(out_d[p, :, c0:c0+512], Osb[:])

    nc.compile()
    return nc


# ---------------------------------------------------------------- PJRT runner
class _SpmdRunner:
    def __init__(self, nc, n_cores):
        import jax
        import numpy as _np
        from jax.sharding import Mesh, PartitionSpec
        from jax.experimental.shard_map import shard_map
        import concourse.mybir as mybir
        from concourse import bass2jax
        bass2jax.install_neuronx_cc_hook()
        self.jax = jax
        self.n_cores = n_cores
        partition_name = (nc.partition_id_tensor.name
                          if nc.partition_id_tensor else None)
        in_names, out_names, out_avals, zero_outs = [], [], [], []
        for alloc in nc.m.functions[0].allocations:
            if not isinstance(alloc, mybir.MemoryLocationSet):
                continue
            name = alloc.memorylocations[0].name
            if alloc.kind == "ExternalInput":
                if name != partition_name:
                    in_names.append(name)
            elif alloc.kind == "ExternalOutput":
                shape = tuple(alloc.tensor_shape)
                dtype = mybir.dt.np(alloc.dtype)
                out_names.append(name)
                out_avals.append(jax.core.ShapedArray(shape, dtype))
                zero_outs.append(_np.zeros(shape, dtype))
        self.in_names, self.out_names = in_names, out_names
        self.out_avals, self.zero_outs = out_avals, zero_outs
        n_params, n_outs = len(in_names), len(out_avals)
        all_in = in_names + out_names
        if partition_name is not None:
            all_in.append(partition_name)

        def _body(*args):
            operands = list(args)
            if partition_name is not None:
                operands.append(bass2jax.partition_id_tensor())
            return tuple(bass2jax._bass_exec_p.bind(
                *operands, out_avals=tuple(out_avals),
                in_names=tuple(all_in), out_names=tuple(out_names),
                lowering_input_output_aliases=(),
                sim_require_finite=True, sim_require_nnan=True, nc=nc))

        devices = jax.devices()[:n_cores]
        mesh = Mesh(_np.asarray(devices), ("core",))
        self.sharded = jax.jit(
            shard_map(_body, mesh=mesh,
                      in_specs=(PartitionSpec("core"),) * (n_params + n_outs),
                      out_specs=(PartitionSpec("core"),) * n_outs,
                      check_rep=False),
            donate_argnums=tuple(range(n_params, n_params + n_outs)),
            keep_unused=True)

    def concat_inputs(self, in_maps):
        import numpy as _np
        per_core = [[_np.asarray(m[n]) for n in self.in_names] for m in in_maps]
        return [_np.concatenate([per_core[c][i] for c in range(self.n_cores)], 0)
                for i in range(len(self.in_names))]

    def zeros(self):
        import numpy as _np
        return [_np.zeros((self.n_cores * z.shape[0], *z.shape[1:]), z.dtype)
                for z in self.zero_outs]

    def run_concat(self, concat_in):
        out_arrs = self.sharded(*concat_in, *self.zeros())
        import numpy as _np
        return [_np.asarray(a) for a in out_arrs]


def _get_runner(rows=R, rep=1):
    key = (rows, rep)
    if key not in _CACHE:
        nc = build_nc(rows, rep)
        _CACHE[key] = _SpmdRunner(nc, N_CORES)
    return _CACHE[key]


# ---------------------------------------------------------------- public entry
def kernel(**inputs):
    ctx_full = np.asarray(inputs['context'], np.float32)
    assert ctx_full.shape == (B, D)
    runner = _get_runner(R, 1)
    in_maps = make_in_maps(inputs)
    concat_in = runner.concat_inputs(in_maps)
    outs = runner.run_concat(concat_in)
    o = outs[0].reshape(N_CORES, NT, 16, R)
    logits = np.ascontiguousarray(o.transpose(0, 3, 1, 2)).reshape(B, NT, 16)
    return logits.astype(np.float32)


# revision 10
# speedup vs baseline: 1.1809x; 1.0034x over previous
"""Trainium2 Bass kernel for nn_AutoregressiveLAMDecoder (B=16384, D=1024, H=8, NT=4, NC=16).

Data-parallel over 8 cores (R=2048 rows/core). Exact algebraic restructure:
  - cross-attention collapses (softmax over one key): ca = mem @ (Wo_ca@Wv_ca).T + b
  - the whole self-attention block output + token/pos embedding depends only on
    the discrete token prefix (<= 4096 combos per position): precomputed on host
    into per-position BASE tables, gathered per row on device (transposed layout)
  - ff_w2/out_w fold into a (2048, 16) matrix; ln1/ln3 affines fold into weights
  - layernorms computed in the transposed (feature-partition) layout via
    ones-vector matmul reductions; no 128x128 transposes anywhere
All matmuls bf16 with fp32 PSUM accumulation; stats and softmax f32 on host.
"""
import sys
for _p in ('/opt/trn_rl_repo', '/root/.axon_site/_ro/trn_rl_repo'):
    if _p not in sys.path:
        sys.path.insert(0, _p)

import math
import numpy as np
import ml_dtypes

B, D, H = 16384, 1024, 8
NT, NC = 4, 16
DFF = 2048
DH = D // H
N_CORES = 8
R = B // N_CORES          # rows per core
BF16 = ml_dtypes.bfloat16

_CACHE = {}


# ---------------------------------------------------------------- host math
def _ln_rows(x, g, b, eps=1e-5):
    m = x.mean(-1, keepdims=True)
    v = ((x - m) ** 2).mean(-1, keepdims=True)
    return (x - m) / np.sqrt(v + eps) * g + b


def _host_precompute(i):
    """Weight-only folds and per-combo BASE tables (float64/float32)."""
    f = {k: np.asarray(v, np.float64) for k, v in i.items()
         if np.asarray(v).dtype not in (np.int64, np.int32)}
    P = {}
    P['WcpT'] = (f['cp_w'] * f['cp_ln_g'][None, :]).T            # (din, dout)
    P['b_cp'] = f['cp_b'] + f['cp_w'] @ f['cp_ln_b']
    W_ca = f['ca_wo'] @ f['ca_wv']
    P['WcaT'] = W_ca.T
    P['b_ca'] = f['ca_wo'] @ f['ca_bv'] + f['ca_bo']
    P['W2pT'] = (f['out_w'] @ f['ff_w2']).T                       # (2048, 16)
    P['b_out2'] = f['out_b'] + f['out_w'] @ f['ff_b2']
    P['W1T'] = (f['ff_w1'] * f['ln3_g'][None, :]).T               # (1024, 2048)
    P['b1'] = f['ff_b1'] + f['ff_w1'] @ f['ln3_b']
    P['OwT'] = f['out_w'].T                                       # (1024, 16)

    E = np.stack([f['tok_emb'] + f['pos_emb'][p][None, :] for p in range(NT)])
    L = np.stack([_ln_rows(E[p], f['ln1_g'], f['ln1_b']) for p in range(NT)])
    Q = (L @ f['sa_wq'].T + f['sa_bq']).reshape(NT, NC + 1, H, DH)
    K = (L @ f['sa_wk'].T + f['sa_bk']).reshape(NT, NC + 1, H, DH)
    V = (L @ f['sa_wv'].T + f['sa_bv']).reshape(NT, NC + 1, H, DH)

    # BASE_p[combo] = E[p][qtok] + SA_out(combo) @ wo.T + bo, where combo
    # encodes targets t_0..t_{p-1} (position p attends to shifted[0..p]).
    bases = []
    for p in range(NT):
        S = NC ** p
        digits = np.arange(S)
        ctoks = np.empty((S, p + 1), np.int64)
        ctoks[:, 0] = NC                                      # start token
        for j in range(1, p + 1):
            ctoks[:, j] = (digits // (NC ** (p - j))) % NC    # t_{j-1}
        qtok = ctoks[:, p]
        # scores s[n, h, j] = Q[p][qtok]·K[j][ctoks_j] / sqrt(dh)
        s = np.empty((S, H, p + 1))
        for j in range(p + 1):
            s[:, :, j] = np.einsum('nhd,nhd->nh', Q[p][qtok],
                                   K[j][ctoks[:, j]]) / math.sqrt(DH)
        s -= s.max(-1, keepdims=True)
        a = np.exp(s)
        a /= a.sum(-1, keepdims=True)
        o = np.zeros((S, H, DH))
        for j in range(p + 1):
            o += a[:, :, j:j+1] * V[j][ctoks[:, j]]
        sa = o.reshape(S, D) @ f['sa_wo'].T + f['sa_bo']
        bases.append((E[p][qtok] + sa).astype(np.float32))
    P['base0'] = bases[0][0]                                  # (1024,)
    P['base1'] = bases[1]                                     # (16, 1024)
    P['base2'] = bases[2]                                     # (256, 1024)
    P['base3'] = bases[3]                                     # (4096, 1024)
    return P


def _shared_inputs(P):
    bf = lambda a: np.ascontiguousarray(a, BF16)
    f32 = lambda a: np.ascontiguousarray(a, np.float32)
    col = lambda b, n: f32(np.asarray(b).reshape(n, 128).T)   # [128, n]
    return {
        'wcp': bf(P['WcpT']), 'wca': bf(P['WcaT']),
        'w1': bf(P['W1T']), 'w2p': bf(P['W2pT']), 'oww': bf(P['OwT']),
        'base1': bf(P['base1']), 'base2': bf(P['base2']),
        'base3': bf(P['base3']),
        'bcp': col(P['b_cp'], 8),
        'bca': col(P['b_ca'], 8),
        'b0c': col(P['base0'], 8),
        'bb1': col(P['b1'], 16),
        'bout': f32(np.asarray(P['b_out2']).reshape(16, 1)),
    }


def _per_core_inputs(P, ctx_shard, tg_shard):
    """Batch-dependent marshalling for one core."""
    r = ctx_shard.shape[0]
    t0 = tg_shard[:, 0].astype(np.int64)
    t1 = tg_shard[:, 1].astype(np.int64)
    t2 = tg_shard[:, 2].astype(np.int64)
    idxs = {'gi1': t0, 'gi2': t0 * 16 + t1, 'gi3': t0 * 256 + t1 * 16 + t2}
    out = {'ctxT': np.ascontiguousarray(ctx_shard.T, dtype=BF16)}
    s16 = np.arange(32) * 16
    for k, idx in idxs.items():
        w = np.zeros((128, r // 16), np.int16)
        for c in range(r // 512):
            blk = idx[c*512:(c+1)*512]
            for q in range(128):
                w[q, c*32:(c+1)*32] = blk[s16 + q % 16]
        out[k] = w
    return out


def make_in_maps(inputs):
    ctx_full = np.asarray(inputs['context'], np.float32)
    tg_full = np.asarray(inputs['targets']).astype(np.int64)
    P = _host_precompute(inputs)
    shared = _shared_inputs(P)
    in_maps = []
    for c in range(N_CORES):
        m = dict(shared)
        m.update(_per_core_inputs(P, ctx_full[c*R:(c+1)*R],
                                  tg_full[c*R:(c+1)*R]))
        in_maps.append(m)
    return in_maps


# ---------------------------------------------------------------- device build
def build_nc(rows=R, rep=1):
    import concourse.bass as bass
    import concourse.mybir as mybir
    from concourse import bacc
    from concourse.tile import TileContext

    dt = mybir.dt
    AF = mybir.ActivationFunctionType
    OP = mybir.AluOpType

    NCH = rows // 512

    nc = bacc.Bacc("TRN2", target_bir_lowering=False, debug=False,
                   num_devices=N_CORES)
    din = lambda n, s, d: nc.dram_tensor(n, s, d, kind="ExternalInput").ap()
    ctxT_d = din("ctxT", [D, rows], dt.bfloat16)
    gi1_d = din("gi1", [128, rows // 16], dt.int16)
    gi2_d = din("gi2", [128, rows // 16], dt.int16)
    gi3_d = din("gi3", [128, rows // 16], dt.int16)
    wcp_d = din("wcp", [D, D], dt.bfloat16)
    wca_d = din("wca", [D, D], dt.bfloat16)
    w1_d = din("w1", [D, DFF], dt.bfloat16)
    w2p_d = din("w2p", [DFF, 16], dt.bfloat16)
    ow_d = din("oww", [D, 16], dt.bfloat16)
    base1_d = din("base1", [16, D], dt.bfloat16)
    base2_d = din("base2", [256, D], dt.bfloat16)
    base3_d = din("base3", [4096, D], dt.bfloat16)
    bcp_d = din("bcp", [128, 8], dt.float32)
    bca_d = din("bca", [128, 8], dt.float32)
    b0c_d = din("b0c", [128, 8], dt.float32)
    bb1_d = din("bb1", [128, 16], dt.float32)
    bout_d = din("bout", [16, 1], dt.float32)
    out_d = nc.dram_tensor("out", [NT, 16, rows], dt.float32,
                           kind="ExternalOutput").ap()
    base_d = [None, base1_d, base2_d, base3_d]
    gi_d = [None, gi1_d, gi2_d, gi3_d]

    with TileContext(nc) as tc:
        with (
            tc.tile_pool(name="wp", bufs=1) as wp,
            tc.tile_pool(name="bt", bufs=3) as btp,
            tc.tile_pool(name="fm", bufs=2) as fm,
            tc.tile_pool(name="rl", bufs=2) as rl,
            tc.tile_pool(name="st", bufs=2) as st,
            tc.tile_pool(name="pmm", bufs=5, space="PSUM") as pmm,
            tc.tile_pool(name="pst", bufs=2, space="PSUM") as pst,
            tc.tile_pool(name="pO", bufs=1, space="PSUM") as pO,
        ):
            # ---- constants / weights
            ones_k = wp.tile([128, 1], dt.bfloat16, tag="onesk")
            nc.vector.memset(ones_k, 1.0)
            ones_m = wp.tile([1, 128], dt.bfloat16, tag="onesm")
            nc.vector.memset(ones_m, 1.0)
            eps1 = wp.tile([1, 1], dt.float32, tag="eps1")
            nc.vector.memset(eps1, 1e-5)

            wcp = wp.tile([128, 8, D], dt.bfloat16, tag="wcp")
            nc.sync.dma_start(wcp[:], wcp_d.rearrange("(k p) n -> p k n", p=128))
            wca = wp.tile([128, 8, D], dt.bfloat16, tag="wca")
            nc.sync.dma_start(wca[:], wca_d.rearrange("(k p) n -> p k n", p=128))
            w1 = wp.tile([128, 8, DFF], dt.bfloat16, tag="w1")
            nc.sync.dma_start(w1[:], w1_d.rearrange("(k p) n -> p k n", p=128))
            w2p = wp.tile([128, 16, 16], dt.bfloat16, tag="w2p")
            nc.sync.dma_start(w2p[:], w2p_d.rearrange("(k p) n -> p k n", p=128))
            oww = wp.tile([128, 8, 16], dt.bfloat16, tag="oww")
            nc.sync.dma_start(oww[:], ow_d.rearrange("(k p) n -> p k n", p=128))
            bcp = wp.tile([128, 8], dt.float32, tag="bcp")
            nc.sync.dma_start(bcp[:], bcp_d[:])
            bca = wp.tile([128, 8], dt.float32, tag="bca")
            nc.sync.dma_start(bca[:], bca_d[:])
            b0c = wp.tile([128, 8], dt.float32, tag="b0c")
            nc.sync.dma_start(b0c[:], b0c_d[:])
            bb1 = wp.tile([128, 16], dt.float32, tag="bb1")
            nc.sync.dma_start(bb1[:], bb1_d[:])
            bout = wp.tile([16, 1], dt.float32, tag="bout")
            nc.sync.dma_start(bout[:], bout_d[:])
            gi = [None] * 4
            for p in (1, 2, 3):
                gtile = wp.tile([128, rows // 16], dt.int16, tag=f"gi{p}")
                nc.sync.dma_start(gtile[:], gi_d[p][:])
                gi[p] = gtile

            def row_stats(src_tiles, sq_tag):
                """src: list of 8 [128,512] bf16 tiles (or [128,8,512] views).
                Returns (mu_b, nmu_b, rs_b) bf16 [128,512] broadcast tiles."""
                sps = pst.tile([1, 512], dt.float32, tag="stat")
                qps = pst.tile([1, 512], dt.float32, tag="stat")
                for kb in range(8):
                    xt = src_tiles(kb)
                    nc.tensor.matmul(sps[:], ones_k[:], xt,
                                     start=(kb == 0), stop=(kb == 7))
                    sq = rl.tile([128, 512], dt.bfloat16, tag=sq_tag, bufs=2)
                    nc.vector.tensor_tensor(sq[:], xt, xt, OP.mult)
                    nc.tensor.matmul(qps[:], ones_k[:], sq[:],
                                     start=(kb == 0), stop=(kb == 7))
                mean = st.tile([1, 512], dt.float32, tag="statf", bufs=3)
                nc.scalar.activation(mean[:], sps[:], AF.Copy, bias=0.0,
                                     scale=1.0 / D)
                meanb = st.tile([1, 512], dt.bfloat16, tag="statb", bufs=2)
                nc.vector.tensor_copy(meanb[:], mean[:])
                m2 = st.tile([1, 512], dt.float32, tag="statf", bufs=3)
                nc.vector.tensor_tensor(m2[:], mean[:], mean[:], OP.mult)
                var = st.tile([1, 512], dt.float32, tag="statf", bufs=3)
                nc.vector.scalar_tensor_tensor(
                    out=var[:], in0=qps[:], scalar=1.0 / D,
                    in1=m2[:], op0=OP.mult, op1=OP.subtract)
                sd = st.tile([1, 512], dt.float32, tag="statf", bufs=3)
                nc.scalar.activation(sd[:], var[:], AF.Sqrt, bias=eps1[:])
                rs = st.tile([1, 512], dt.bfloat16, tag="statb", bufs=2)
                with nc.allow_low_precision(reason="rstd broadcast is bf16 anyway"):
                    nc.vector.reciprocal(rs[:], sd[:])
                mbc = pmm.tile([128, 512], dt.float32, tag="mm")
                nc.tensor.matmul(mbc[:], ones_m[:], meanb[:], start=True,
                                 stop=True)
                rbc = pmm.tile([128, 512], dt.float32, tag="mm")
                nc.tensor.matmul(rbc[:], ones_m[:], rs[:], start=True,
                                 stop=True)
                mu_b = st.tile([128, 512], dt.bfloat16, tag="mub", bufs=2)
                nc.scalar.copy(mu_b[:], mbc[:])
                rs_b = st.tile([128, 512], dt.bfloat16, tag="rsb", bufs=2)
                nc.scalar.copy(rs_b[:], rbc[:])
                return mu_b, rs_b

            for chn in range(NCH):
                c0 = chn * 512
                # ---- load ctxT slice, LN via matmul stats
                xt = fm.tile([128, 8, 512], dt.bfloat16, tag="xt", bufs=1)
                for kb in range(8):
                    nc.sync.dma_start(
                        xt[:, kb, :], ctxT_d[kb*128:(kb+1)*128, c0:c0+512])
                mu_b, rs_b = row_stats(lambda kb: xt[:, kb, :], "sqc")
                lnx = fm.tile([128, 8, 512], dt.bfloat16, tag="lnx", bufs=1)
                for kb in range(8):
                    t = rl.tile([128, 512], dt.bfloat16, tag="t", bufs=2)
                    nc.vector.tensor_tensor(t[:], xt[:, kb, :], mu_b[:],
                                            OP.subtract)
                    nc.vector.tensor_tensor(lnx[:, kb, :], t[:], rs_b[:],
                                            OP.mult)
                # ---- mem = gelu(cp(lnx))
                mem = fm.tile([128, 8, 512], dt.bfloat16, tag="mem", bufs=1)
                for mb in range(8):
                    z = pmm.tile([128, 512], dt.float32, tag="mm")
                    for kb in range(8):
                        nc.tensor.matmul(z[:], wcp[:, kb, mb*128:(mb+1)*128],
                                         lnx[:, kb, :],
                                         start=(kb == 0), stop=(kb == 7))
                    nc.scalar.activation(mem[:, mb, :], z[:], AF.Gelu,
                                         bias=bcp[:, mb:mb+1])
                # ---- ca = Wca @ mem + bca
                casb = fm.tile([128, 8, 512], dt.bfloat16, tag="ca")
                for mb in range(8):
                    z = pmm.tile([128, 512], dt.float32, tag="mm")
                    for kb in range(8):
                        nc.tensor.matmul(z[:], wca[:, kb, mb*128:(mb+1)*128],
                                         mem[:, kb, :],
                                         start=(kb == 0), stop=(kb == 7))
                    nc.scalar.activation(casb[:, mb, :], z[:], AF.Identity,
                                         bias=bca[:, mb:mb+1])

                def emit_head(p):
                    # x2 = base_p(combo) + ca; ln3 stats; normalize
                    x2 = fm.tile([128, 8, 512], dt.bfloat16, tag="x2", bufs=2)
                    if p == 0:
                        for kb in range(8):
                            nc.vector.tensor_scalar(
                                x2[:, kb, :], casb[:, kb, :],
                                b0c[:, kb:kb+1], None, OP.add)
                    else:
                        bt = btp.tile([128, 8, 512], dt.bfloat16, tag="bt")
                        nc.gpsimd.dma_gather(
                            out_ap=bt[:],
                            in_ap=base_d[p],
                            idxs_ap=gi[p][:, chn*32:(chn+1)*32],
                            num_idxs=512,
                            num_idxs_reg=512,
                            elem_size=D,
                            transpose=True,
                        )
                        for kb in range(8):
                            nc.vector.tensor_tensor(
                                x2[:, kb, :], bt[:, kb, :], casb[:, kb, :],
                                OP.add)
                    mu3, rs3 = row_stats(lambda kb: x2[:, kb, :], "sq2")
                    x2n = fm.tile([128, 8, 512], dt.bfloat16, tag="x2n", bufs=2)
                    for kb in range(8):
                        t = rl.tile([128, 512], dt.bfloat16, tag="t", bufs=2)
                        nc.vector.tensor_tensor(t[:], x2[:, kb, :], mu3[:],
                                                OP.subtract)
                        nc.vector.tensor_tensor(x2n[:, kb, :], t[:], rs3[:],
                                                OP.mult)
                    return x2, x2n

                head = emit_head(0)
                for p in range(NT):
                    x2, x2n = head
                    if p + 1 < NT:
                        head = emit_head(p + 1)
                    # ---- out = x2 @ Ow + relu(x2n @ W1 + b1) @ W2p + bout
                    Ops = pO.tile([16, 512], dt.float32, tag="O")
                    for kb in range(8):
                        nc.tensor.matmul(Ops[:], oww[:, kb, :], x2[:, kb, :],
                                         start=(kb == 0), stop=False)
                    for fb in range(16):
                        hps = pmm.tile([128, 512], dt.float32, tag="mm")
                        for kb in range(8):
                            nc.tensor.matmul(
                                hps[:], w1[:, kb, fb*128:(fb+1)*128],
                                x2n[:, kb, :],
                                start=(kb == 0), stop=(kb == 7))
                        hsb = rl.tile([128, 512], dt.bfloat16, tag="hsb",
                                      bufs=2)
                        nc.scalar.activation(hsb[:], hps[:], AF.Relu,
                                             bias=bb1[:, fb:fb+1])
                        nc.tensor.matmul(Ops[:], w2p[:, fb, :], hsb[:],
                                         start=False, stop=(fb == 15))
                    Osb = rl.tile([16, 512], dt.float32, tag="Osb", bufs=2)
                    nc.scalar.activation(Osb[:], Ops[:], AF.Identity,
                                         bias=bout[:, 0:1])
                    nc.sync.dma_start(out_d[p, :, c0:c0+512], Osb[:])

    nc.compile()
    return nc


# ---------------------------------------------------------------- PJRT runner
class _SpmdRunner:
    def __init__(self, nc, n_cores):
        import jax
        import numpy as _np
        from jax.sharding import Mesh, PartitionSpec
        from jax.experimental.shard_map import shard_map
        import concourse.mybir as mybir
        from concourse import bass2jax
        bass2jax.install_neuronx_cc_hook()
        self.jax = jax
        self.n_cores = n_cores
        partition_name = (nc.partition_id_tensor.name
                          if nc.partition_id_tensor else None)
        in_names, out_names, out_avals, zero_outs = [], [], [], []
        for alloc in nc.m.functions[0].allocations:
            if not isinstance(alloc, mybir.MemoryLocationSet):
                continue
            name = alloc.memorylocations[0].name
            if alloc.kind == "ExternalInput":
                if name != partition_name:
                    in_names.append(name)
            elif alloc.kind == "ExternalOutput":
                shape = tuple(alloc.tensor_shape)
                dtype = mybir.dt.np(alloc.dtype)
                out_names.append(name)
                out_avals.append(jax.core.ShapedArray(shape, dtype))
                zero_outs.append(_np.zeros(shape, dtype))
        self.in_names, self.out_names = in_names, out_names
        self.out_avals, self.zero_outs = out_avals, zero_outs
        n_params, n_outs = len(in_names), len(out_avals)
        all_in = in_names + out_names
        if partition_name is not None:
            all_in.append(partition_name)

        def _body(*args):
            operands = list(args)
            if partition_name is not None:
                operands.append(bass2jax.partition_id_tensor())
            return tuple(bass2jax._bass_exec_p.bind(
                *operands, out_avals=tuple(out_avals),
                in_names=tuple(all_in), out_names=tuple(out_names),
                lowering_input_output_aliases=(),
                sim_require_finite=True, sim_require_nnan=True, nc=nc))

        devices = jax.devices()[:n_cores]
        mesh = Mesh(_np.asarray(devices), ("core",))
        self.sharded = jax.jit(
            shard_map(_body, mesh=mesh,
                      in_specs=(PartitionSpec("core"),) * (n_params + n_outs),
                      out_specs=(PartitionSpec("core"),) * n_outs,
                      check_rep=False),
            donate_argnums=tuple(range(n_params, n_params + n_outs)),
            keep_unused=True)

    def concat_inputs(self, in_maps):
        import numpy as _np
        per_core = [[_np.asarray(m[n]) for n in self.in_names] for m in in_maps]
        return [_np.concatenate([per_core[c][i] for c in range(self.n_cores)], 0)
                for i in range(len(self.in_names))]

    def zeros(self):
        import numpy as _np
        return [_np.zeros((self.n_cores * z.shape[0], *z.shape[1:]), z.dtype)
                for z in self.zero_outs]

    def run_concat(self, concat_in):
        out_arrs = self.sharded(*concat_in, *self.zeros())
        import numpy as _np
        return [_np.asarray(a) for a in out_arrs]


def _get_runner(rows=R, rep=1):
    key = (rows, rep)
    if key not in _CACHE:
        nc = build_nc(rows, rep)
        _CACHE[key] = _SpmdRunner(nc, N_CORES)
    return _CACHE[key]


# ---------------------------------------------------------------- public entry
def kernel(**inputs):
    ctx_full = np.asarray(inputs['context'], np.float32)
    assert ctx_full.shape == (B, D)
    runner = _get_runner(R, 1)
    in_maps = make_in_maps(inputs)
    concat_in = runner.concat_inputs(in_maps)
    outs = runner.run_concat(concat_in)
    o = outs[0].reshape(N_CORES, NT, 16, R)
    logits = np.ascontiguousarray(o.transpose(0, 3, 1, 2)).reshape(B, NT, 16)
    return logits.astype(np.float32)
